# revision 21
# baseline (speedup 1.0000x reference)
"""BinnedColorLoss Trainium2 kernel (v3: class-sorted one-hot matmul, fp8).

loss = -mean_{b,h,w}[ (sum_k logp[b, idx_k, h, w] * wts_k) * w ]
with logp = log_softmax(pred, axis=1), idx/wts/w gathered per-pixel from
313-entry KNN tables via the pixel's bin t = binned_color[b,0,h,w].

Math restructuring (per pixel, t = bin, lse = logsumexp over C):
  sum_k logp[idx_k]*wts_k*w = (sum_k pred[idx_k]*wts_k)*w - lse * (w*sum_k wts_k)
Summing over all pixels:
  G  = sum_{t,c} A[t,c] * S[t,c],   A[t,c] = w[t]*sum_k wts[t,k]*[idx[t,k]=c]
                                    S[t,c] = sum_{pix in bin t} pred[c,pix]
  L  = sum_pix lse(pix) * coef(t(pix)),   coef[t] = w[t]*sum_k wts[t,k]
  loss = (L - G) / N

Device strategy (data-parallel over 8 cores, 2 images each):
  - Host groups each core's 32768 pixels into 3 bin-classes (t//128) so a
    128-pixel chunk's bins live in ONE 128-wide window -> a single one-hot
    matmul per chunk (vs 3 unsorted).  Pixel order within a class is free
    (the loss is a plain sum over pixels).
  - Host ships pred pixel-major fp8e4 [128, NCHUNK, 320] (cols 313:320
    filled with -240 so exp()->0), a precomputed fp8 one-hot
    [128, NCHUNK, 128] (on-device is_equal via per-partition scalar is a
    pathological DVE/GpSimd path), coef [128, NCHUNK] f32 and the folded
    A table [128, 3, 313] f32.
  - ACT: exp over each group's [128, G*320] fp8->bf16 (no max subtraction:
    inputs are ~N(0,1); fp8 caps |x|<=240, exp stays finite).
  - DVE: grouped pairwise-add tree (bf16 2x mode) 320->160->80->40->20,
    then one tensor_reduce -> sumexp [128, G] per group.
    (tensor_tensor_reduce crashes the NRT exec unit on this stack - avoid.)
  - PE:  S_m[tl, :] += OH_chunk[:, 0:sz].T @ pred_chunk[:, 0:313]  (fp8,
    PSUM f32, one accumulation group per class).
  - Tail: lse = ln(sumexp); L = <lse, coef>; G_m = <S_m, A_m> via
    tensor_mul + tensor_reduce -> out [128, 4].
Host combines the 8 per-core [128,4] partials: loss = (L - G)/N.
"""

import sys

for _p in ("/opt/trn_rl_repo",):
    if _p not in sys.path:
        sys.path.insert(0, _p)

from contextlib import ExitStack

import numpy as np

import concourse.bacc as bacc
import concourse.bass as bass  # noqa: F401  (engine namespaces live on the nc)
import concourse.mybir as mybir
from concourse import bass_utils, tile

F32 = mybir.dt.float32
BF16 = mybir.dt.bfloat16
F8 = mybir.dt.float8e4

B, C, H, W, K = 16, 313, 128, 128, 5
CP = 313                   # channel count as stored (no pad columns)
NCORES = 8
BPC = B // NCORES          # images per core
PIX = BPC * H * W          # pixels per core (32768)
P = 128                    # pixels per chunk (partition dim)
NTOT = B * H * W           # mean denominator
CLS_OFF = (0, 128, 256)    # class windows along t
CLS_SZ = (128, 128, 57)
ADD = mybir.AluOpType.add
MULT = mybir.AluOpType.mult


BLK = 72                   # chunks per DMA super-tile (23KB descriptors)
EG = 24                    # max chunks per exp/tree group


def _plan(nreal):
    """Round nreal up to a multiple of EG; return (nchunk, blocks, groups).

    blocks: list of (tile_start, tile_len) DMA super-tiles.
    groups: list of (tile_idx, off, len) exp groups; the first tile is
    filled by a ramp of small DMAs so the pipeline starts early, and the
    last group is small so the serial tail tree is short.
    """
    nchunk = nreal
    blocks = []
    pos = 0
    while pos < nchunk:
        ln = min(BLK, nchunk - pos)
        blocks.append((pos, ln))
        pos += ln
    return nchunk, blocks


def _tail_fills(rem):
    """Split the last block into even fills ending small (short tail tree)."""
    out = []
    while rem > 24:
        f = min(24, rem - 10)
        f -= f % 2
        out.append(f)
        rem -= f
    if rem > 12:
        out += [rem - 10, 6, 4]
    elif rem > 4:
        out += [rem - 4, 4]
    else:
        out += [rem]
    assert sum(out) == rem if not out else True
    return out


def build_program(ncls, nchunk, blocks):
    """ncls: real (even) chunk counts per class; sum(ncls) <= nchunk."""
    nreal = sum(ncls)
    starts = [0, ncls[0], ncls[0] + ncls[1]]
    ends = [ncls[0], ncls[0] + ncls[1], nreal]

    def cls_of(ch):
        return 0 if ch < ends[0] else (1 if ch < ends[1] else 2)

    nc = bacc.Bacc(
        "TRN2",
        target_bir_lowering=False,
        debug=False,
        enable_asserts=False,
        num_devices=NCORES,
    )
    pred_d = nc.dram_tensor("pred_pm", [P, nchunk, CP], F8, kind="ExternalInput").ap()
    oh_d = nc.dram_tensor("oh", [P, nchunk, P], F8, kind="ExternalInput").ap()
    coef_d = nc.dram_tensor("coef", [P, nchunk], F32, kind="ExternalInput").ap()
    atab_d = nc.dram_tensor("atab", [P, 3, C], F32, kind="ExternalInput").ap()
    out_d = nc.dram_tensor("out", [P, 4], F32, kind="ExternalOutput").ap()

    with tile.TileContext(nc) as tc, ExitStack() as ctx:
        const = ctx.enter_context(tc.tile_pool(name="const", bufs=1))
        predp = ctx.enter_context(tc.tile_pool(name="pred", bufs=3))
        ohp = ctx.enter_context(tc.tile_pool(name="oh", bufs=3))
        expp = ctx.enter_context(tc.tile_pool(name="exp", bufs=3))
        scrp = ctx.enter_context(tc.tile_pool(name="scr", bufs=2))
        accp = ctx.enter_context(tc.tile_pool(name="acc", bufs=1))
        psum = ctx.enter_context(tc.tile_pool(name="psum", bufs=1, space="PSUM"))

        # Preload the one ACT table set containing BOTH Exp and Ln so the
        # compiler's per-function table loads (one of which lands on the
        # serial tail, 1.3us) are never needed.
        try:
            from concourse.hw_specs import get_activation_tables

            _tabs = list(get_activation_tables(nc.m.arch).items())
            _EXP = mybir.ActivationFunctionType.Exp
            _LN = mybir.ActivationFunctionType.Ln
            _set_id = next(
                i for i, (_nm, fns) in enumerate(_tabs)
                if _EXP in fns and _LN in fns
            )
            nc.scalar.add_instruction(
                mybir.InstLoadActFuncSet(
                    name=nc.get_next_instruction_name(),
                    act_func_set_id=_set_id,
                    ins=[],
                    outs=[],
                )
            )
        except Exception:
            pass

        sume_t = accp.tile([P, nchunk], F32, tag="sume")
        out_t = accp.tile([P, 4], F32, tag="out")
        nc.vector.memset(out_t[:], 0.0)
        coef_t = const.tile([P, nchunk], F32, tag="coef")
        nc.gpsimd.dma_start(coef_t[:], coef_d)
        atab_t = const.tile([P, 3, C], F32, tag="atab")
        nc.gpsimd.dma_start(atab_t[:], atab_d)

        spsum = [
            psum.tile([P, C], F32, tag=f"sacc{m}", name=f"sacc{m}")
            for m in range(3)
        ]
        scrg = [
            accp.tile([P, C], F32, tag=f"scrg{m}", name=f"scrg{m}")
            for m in range(3)
        ]

        def emit_gdot(m):
            """<S_m, A_m> -> out_t col 1+m (runs on DVE once class m closed)."""
            sz = CLS_SZ[m]
            nc.vector.tensor_tensor(
                scrg[m][0:sz, :], spsum[m][0:sz, :], atab_t[0:sz, m, :], op=MULT
            )
            nc.vector.tensor_reduce(
                out_t[0:sz, 1 + m:2 + m], scrg[m][0:sz, :],
                axis=mybir.AxisListType.X, op=ADD,
            )

        closed = [False, False, False]
        n_emitted = 0
        for bi, (b0, blen) in enumerate(blocks):
            pt = predp.tile([P, BLK, CP], F8, tag="pred")
            oht = ohp.tile([P, BLK, P], F8, tag="oh")
            # Ramp the first tile with small partial-fill DMAs so exp starts
            # early; split the last tile so the tail tree is short.
            if bi == 0:
                fills = [4, 8, 12, 24, 24]
                assert blen == 72
            elif bi == len(blocks) - 1:
                fills = _tail_fills(blen)
            else:
                fills = [24] * (blen // 24)
            assert sum(fills) == blen and all(f % 2 == 0 for f in fills)
            off = 0
            for fl in fills:
                sl = slice(off, off + fl)
                # pred rides the sync/HWDGE ring, oh+consts the gpsimd/
                # SWDGE ring (splitting either stream across rings measured
                # slower).  ACT never issues DMAs (it is the bottleneck).
                nc.sync.dma_start(pt[:, sl, :], pred_d[:, b0 + off:b0 + off + fl, :])
                nc.gpsimd.dma_start(oht[:, sl, :], oh_d[:, b0 + off:b0 + off + fl, :])
                et = expp.tile([P, EG, CP], BF16, tag="exp")
                nc.scalar.activation(
                    et[:, 0:fl, :], pt[:, sl, :],
                    mybir.ActivationFunctionType.Exp,
                )
                c0 = b0 + off
                if fl <= 6:
                    # tiny tail fills: one flat reduce beats a sem-chained
                    # tree
                    nc.vector.tensor_reduce(
                        sume_t[:, c0:c0 + fl], et[:, 0:fl, :],
                        axis=mybir.AxisListType.X, op=ADD,
                    )
                else:
                    s1 = scrp.tile([P, EG, 156], BF16, tag="s1")
                    nc.vector.tensor_tensor(
                        s1[:, 0:fl, :], et[:, 0:fl, 0:156],
                        et[:, 0:fl, 156:312], op=ADD)
                    s2 = scrp.tile([P, EG, 78], BF16, tag="s2")
                    nc.vector.tensor_tensor(
                        s2[:, 0:fl, :], s1[:, 0:fl, 0:78], s1[:, 0:fl, 78:156],
                        op=ADD)
                    s3 = scrp.tile([P, EG, 39], BF16, tag="s3")
                    nc.vector.tensor_tensor(
                        s3[:, 0:fl, :], s2[:, 0:fl, 0:39], s2[:, 0:fl, 39:78],
                        op=ADD)
                    # fold the odd straggler column 312 into s3 col 0
                    nc.vector.tensor_tensor(
                        s3[:, 0:fl, 0:1], s3[:, 0:fl, 0:1],
                        et[:, 0:fl, 312:313], op=ADD)
                    nc.vector.tensor_reduce(
                        sume_t[:, c0:c0 + fl], s3[:, 0:fl, :],
                        axis=mybir.AxisListType.X, op=ADD,
                    )

                # DoubleRow fp8: one matmul contracts a PAIR of chunks
                # (256 px). Class counts are even, so pairs never straddle
                # classes.
                for j in range(off, off + fl, 2):
                    ch = b0 + j
                    if ch >= nreal:
                        continue
                    m = cls_of(ch)
                    sz = CLS_SZ[m]
                    nc.tensor.matmul(
                        spsum[m][0:sz, :],
                        oht[:, j:j + 2, 0:sz],
                        pt[:, j:j + 2, 0:C],
                        start=(ch == starts[m]),
                        stop=(ch >= ends[m] - 2),
                        perf_mode=mybir.MatmulPerfMode.DoubleRow,
                    )
                off += fl
                n_emitted += fl
                for m in range(3):
                    if not closed[m] and n_emitted >= ends[m]:
                        emit_gdot(m)
                        closed[m] = True

        for m in range(3):
            if not closed[m]:
                emit_gdot(m)

        lse_t = accp.tile([P, nchunk], F32, tag="lse")
        nc.scalar.activation(
            lse_t[:], sume_t[:], mybir.ActivationFunctionType.Ln
        )
        scrl = accp.tile([P, nchunk], F32, tag="scrl")
        nc.vector.tensor_tensor(scrl[:], lse_t[:], coef_t[:], op=MULT)
        nc.vector.tensor_reduce(
            out_t[:, 0:1], scrl[:], axis=mybir.AxisListType.X, op=ADD
        )
        nc.sync.dma_start(out_d, out_t[:])

    nc.compile()
    return nc


def host_inputs(pred, binned_color, knn_idx, knn_weights, weights):
    """Returns (in_maps, ncls, g, nchunk)."""
    import ml_dtypes

    f8 = ml_dtypes.float8_e4m3
    pred = np.asarray(pred, dtype=np.float32)
    binned = np.asarray(binned_color)
    knn_idx = np.asarray(knn_idx).astype(np.int64)
    knn_w = np.asarray(knn_weights, dtype=np.float32)
    wts = np.asarray(weights, dtype=np.float32)

    # A[t, c] = w[t] * sum_k knn_w[t,k] * [knn_idx[t,k] == c]
    a_full = np.zeros((C, C), dtype=np.float32)
    rows = np.repeat(np.arange(C), K)
    cols = knn_idx.reshape(-1)
    vals = (wts[:, None] * knn_w).reshape(-1)
    np.add.at(a_full, (rows, cols), vals)
    atab = np.zeros((P, 3, C), dtype=np.float32)
    for m in range(3):
        atab[0:CLS_SZ[m], m, :] = a_full[CLS_OFF[m]:CLS_OFF[m] + CLS_SZ[m], :]

    coef_full = wts * knn_w.sum(axis=1)          # (C,)

    t_all, cls_all, counts_all = [], [], []
    for core in range(NCORES):
        bs = slice(core * BPC, (core + 1) * BPC)
        t = binned[bs, 0].reshape(PIX).astype(np.int64)
        cl = t // P
        t_all.append(t)
        cls_all.append(cl)
        counts_all.append(np.bincount(cl, minlength=3))
    counts_all = np.stack(counts_all)            # (8, 3)
    # even chunk counts per class so DoubleRow pairs stay within one class
    ncls = tuple(
        (int(-(-counts_all[:, m].max() // P)) + 1) // 2 * 2 for m in range(3)
    )
    nchunk, blocks = _plan(sum(ncls))
    base = np.cumsum([0, ncls[0], ncls[1]])[:3] * P

    in_maps = []
    for core in range(NCORES):
        bs = slice(core * BPC, (core + 1) * BPC)
        t, cl, cnt = t_all[core], cls_all[core], counts_all[core]
        order = np.argsort(cl, kind="stable")
        slots = np.empty(PIX, dtype=np.int64)
        pos = 0
        for m in range(3):
            slots[order[pos:pos + cnt[m]]] = base[m] + np.arange(cnt[m])
            pos += cnt[m]

        pm = np.ascontiguousarray(
            pred[bs].transpose(0, 2, 3, 1)
        ).reshape(PIX, C)
        pred_slots = np.zeros((nchunk * P, CP), dtype=f8)
        pred_slots[slots, 0:C] = pm.astype(f8)
        pred_pm = np.ascontiguousarray(
            pred_slots.reshape(nchunk, P, CP).transpose(1, 0, 2)
        )

        oh_u8 = np.zeros((nchunk * P, P), dtype=np.uint8)
        oh_u8[slots, t - cl * P] = 0x38          # fp8e4 1.0
        oh = np.ascontiguousarray(
            oh_u8.reshape(nchunk, P, P).transpose(1, 0, 2)
        ).view(f8)

        coef_slots = np.zeros(nchunk * P, dtype=np.float32)
        coef_slots[slots] = coef_full[t]
        coef = np.ascontiguousarray(coef_slots.reshape(nchunk, P).T)

        in_maps.append(
            {"pred_pm": pred_pm, "oh": oh, "coef": coef, "atab": atab}
        )
    return in_maps, ncls, nchunk, blocks


def combine_outputs(core_outs):
    """core_outs: list of [128, 4] f32 arrays -> scalar loss."""
    total = 0.0
    for o in core_outs:
        o = o.astype(np.float64)
        total += o[:, 0].sum() - o[:, 1:4].sum()
    return np.array(total / NTOT, dtype=np.float32)


_NC_CACHE = {}


def kernel(pred, _color, binned_color, knn_idx, knn_weights, weights):
    in_maps, ncls, nchunk, blocks = host_inputs(
        pred, binned_color, knn_idx, knn_weights, weights
    )
    key = (ncls, nchunk)
    if key not in _NC_CACHE:
        _NC_CACHE[key] = build_program(ncls, nchunk, blocks)
    nc = _NC_CACHE[key]
    res = bass_utils.run_bass_kernel_spmd(nc, in_maps, core_ids=list(range(NCORES)))
    outs = [res.results[i]["out"] for i in range(NCORES)]
    return combine_outputs(outs)


if __name__ == "__main__":
    import jax
    import reference

    with jax.default_device(jax.devices("cpu")[0]):
        inputs = reference.setup_inputs()
        inputs = {k: np.asarray(jax.device_get(v)) for k, v in inputs.items()}
    got = kernel(**inputs)
    print("kernel loss:", got)


# revision 22
# speedup vs baseline: 1.0941x; 1.0941x over previous
"""BinnedColorLoss Trainium2 kernel (v3: class-sorted one-hot matmul, fp8).

loss = -mean_{b,h,w}[ (sum_k logp[b, idx_k, h, w] * wts_k) * w ]
with logp = log_softmax(pred, axis=1), idx/wts/w gathered per-pixel from
313-entry KNN tables via the pixel's bin t = binned_color[b,0,h,w].

Math restructuring (per pixel, t = bin, lse = logsumexp over C):
  sum_k logp[idx_k]*wts_k*w = (sum_k pred[idx_k]*wts_k)*w - lse * (w*sum_k wts_k)
Summing over all pixels:
  G  = sum_{t,c} A[t,c] * S[t,c],   A[t,c] = w[t]*sum_k wts[t,k]*[idx[t,k]=c]
                                    S[t,c] = sum_{pix in bin t} pred[c,pix]
  L  = sum_pix lse(pix) * coef(t(pix)),   coef[t] = w[t]*sum_k wts[t,k]
  loss = (L - G) / N

Device strategy (data-parallel over 8 cores, 2 images each):
  - Host groups each core's 32768 pixels into 3 bin-classes (t//128) so a
    128-pixel chunk's bins live in ONE 128-wide window -> a single one-hot
    matmul per chunk (vs 3 unsorted).  Pixel order within a class is free
    (the loss is a plain sum over pixels).
  - Host ships pred pixel-major fp8e4 [128, NCHUNK, 320] (cols 313:320
    filled with -240 so exp()->0), a precomputed fp8 one-hot
    [128, NCHUNK, 128] (on-device is_equal via per-partition scalar is a
    pathological DVE/GpSimd path), coef [128, NCHUNK] f32 and the folded
    A table [128, 3, 313] f32.
  - ACT: exp over each group's [128, G*320] fp8->bf16 (no max subtraction:
    inputs are ~N(0,1); fp8 caps |x|<=240, exp stays finite).
  - DVE: grouped pairwise-add tree (bf16 2x mode) 320->160->80->40->20,
    then one tensor_reduce -> sumexp [128, G] per group.
    (tensor_tensor_reduce crashes the NRT exec unit on this stack - avoid.)
  - PE:  S_m[tl, :] += OH_chunk[:, 0:sz].T @ pred_chunk[:, 0:313]  (fp8,
    PSUM f32, one accumulation group per class).
  - Tail: lse = ln(sumexp); L = <lse, coef>; G_m = <S_m, A_m> via
    tensor_mul + tensor_reduce -> out [128, 4].
Host combines the 8 per-core [128,4] partials: loss = (L - G)/N.
"""

import sys

for _p in ("/opt/trn_rl_repo",):
    if _p not in sys.path:
        sys.path.insert(0, _p)

from contextlib import ExitStack

import numpy as np

import concourse.bacc as bacc
import concourse.bass as bass  # noqa: F401  (engine namespaces live on the nc)
import concourse.mybir as mybir
from concourse import bass_utils, tile

F32 = mybir.dt.float32
BF16 = mybir.dt.bfloat16
F8 = mybir.dt.float8e4

B, C, H, W, K = 16, 313, 128, 128, 5
CP = 320                   # padded channel count (pad cols = -240 -> exp 0; 4B-aligned rows)
NCORES = 8
BPC = B // NCORES          # images per core
PIX = BPC * H * W          # pixels per core (32768)
P = 128                    # pixels per chunk (partition dim)
NTOT = B * H * W           # mean denominator
CLS_OFF = (0, 128, 256)    # class windows along t
CLS_SZ = (128, 128, 57)
ADD = mybir.AluOpType.add
MULT = mybir.AluOpType.mult


BLK = 72                   # chunks per DMA super-tile (23KB descriptors)
EG = 24                    # max chunks per exp/tree group


def _plan(nreal):
    """Round nreal up to a multiple of EG; return (nchunk, blocks, groups).

    blocks: list of (tile_start, tile_len) DMA super-tiles.
    groups: list of (tile_idx, off, len) exp groups; the first tile is
    filled by a ramp of small DMAs so the pipeline starts early, and the
    last group is small so the serial tail tree is short.
    """
    nchunk = nreal
    blocks = []
    pos = 0
    while pos < nchunk:
        ln = min(BLK, nchunk - pos)
        blocks.append((pos, ln))
        pos += ln
    return nchunk, blocks


def _tail_fills(rem):
    """Split the last block into even fills ending small (short tail tree)."""
    out = []
    while rem > 24:
        f = min(24, rem - 10)
        f -= f % 2
        out.append(f)
        rem -= f
    if rem > 12:
        out += [rem - 10, 6, 4]
    elif rem > 4:
        out += [rem - 4, 4]
    else:
        out += [rem]
    assert sum(out) == rem if not out else True
    return out


def build_program(ncls, nchunk, blocks):
    """ncls: real (even) chunk counts per class; sum(ncls) <= nchunk."""
    nreal = sum(ncls)
    starts = [0, ncls[0], ncls[0] + ncls[1]]
    ends = [ncls[0], ncls[0] + ncls[1], nreal]

    def cls_of(ch):
        return 0 if ch < ends[0] else (1 if ch < ends[1] else 2)

    nc = bacc.Bacc(
        "TRN2",
        target_bir_lowering=False,
        debug=False,
        enable_asserts=False,
        num_devices=NCORES,
    )
    pred_d = nc.dram_tensor("pred_pm", [P, nchunk, CP], F8, kind="ExternalInput").ap()
    oh_d = nc.dram_tensor("oh", [P, nchunk, P], F8, kind="ExternalInput").ap()
    coef_d = nc.dram_tensor("coef", [P, nchunk], F32, kind="ExternalInput").ap()
    atab_d = nc.dram_tensor("atab", [P, 3, C], F32, kind="ExternalInput").ap()
    out_d = nc.dram_tensor("out", [P, 4], F32, kind="ExternalOutput").ap()

    with tile.TileContext(nc) as tc, ExitStack() as ctx:
        const = ctx.enter_context(tc.tile_pool(name="const", bufs=1))
        predp = ctx.enter_context(tc.tile_pool(name="pred", bufs=2))
        ohp = ctx.enter_context(tc.tile_pool(name="oh", bufs=2))
        expp = ctx.enter_context(tc.tile_pool(name="exp", bufs=3))
        scrp = ctx.enter_context(tc.tile_pool(name="scr", bufs=2))
        accp = ctx.enter_context(tc.tile_pool(name="acc", bufs=1))
        psum = ctx.enter_context(tc.tile_pool(name="psum", bufs=1, space="PSUM"))

        # Preload the one ACT table set containing BOTH Exp and Ln so the
        # compiler's per-function table loads (one of which lands on the
        # serial tail, 1.3us) are never needed.
        try:
            from concourse.hw_specs import get_activation_tables

            _tabs = list(get_activation_tables(nc.m.arch).items())
            _EXP = mybir.ActivationFunctionType.Exp
            _LN = mybir.ActivationFunctionType.Ln
            _set_id = next(
                i for i, (_nm, fns) in enumerate(_tabs)
                if _EXP in fns and _LN in fns
            )
            nc.scalar.add_instruction(
                mybir.InstLoadActFuncSet(
                    name=nc.get_next_instruction_name(),
                    act_func_set_id=_set_id,
                    ins=[],
                    outs=[],
                )
            )
        except Exception:
            pass

        sume_t = accp.tile([P, nchunk], F32, tag="sume")
        out_t = accp.tile([P, 4], F32, tag="out")
        nc.vector.memset(out_t[:], 0.0)
        coef_t = const.tile([P, nchunk], F32, tag="coef")
        nc.gpsimd.dma_start(coef_t[:], coef_d)
        atab_t = const.tile([P, 3, C], F32, tag="atab")
        nc.gpsimd.dma_start(atab_t[:], atab_d)

        spsum = [
            psum.tile([P, C], F32, tag=f"sacc{m}", name=f"sacc{m}")
            for m in range(3)
        ]
        scrg = [
            accp.tile([P, C], F32, tag=f"scrg{m}", name=f"scrg{m}")
            for m in range(3)
        ]

        def emit_gdot(m):
            """<S_m, A_m> -> out_t col 1+m (runs on DVE once class m closed)."""
            sz = CLS_SZ[m]
            nc.vector.tensor_tensor(
                scrg[m][0:sz, :], spsum[m][0:sz, :], atab_t[0:sz, m, :], op=MULT
            )
            nc.vector.tensor_reduce(
                out_t[0:sz, 1 + m:2 + m], scrg[m][0:sz, :],
                axis=mybir.AxisListType.X, op=ADD,
            )

        closed = [False, False, False]
        n_emitted = 0
        for bi, (b0, blen) in enumerate(blocks):
            pt = predp.tile([P, BLK, CP], F8, tag="pred")
            oht = ohp.tile([P, BLK, P], F8, tag="oh")
            # Ramp the first tile with small partial-fill DMAs so exp starts
            # early; split the last tile so the tail tree is short.
            if bi == 0:
                fills = [4, 8, 12, 24, 24]
                assert blen == 72
            elif bi == len(blocks) - 1:
                fills = _tail_fills(blen)
            else:
                fills = [24] * (blen // 24)
            assert sum(fills) == blen and all(f % 2 == 0 for f in fills)
            off = 0
            for fl in fills:
                sl = slice(off, off + fl)
                # pred rides the sync/HWDGE ring, oh+consts the gpsimd/
                # SWDGE ring (splitting either stream across rings measured
                # slower).  ACT never issues DMAs (it is the bottleneck).
                nc.sync.dma_start(pt[:, sl, :], pred_d[:, b0 + off:b0 + off + fl, :])
                nc.gpsimd.dma_start(oht[:, sl, :], oh_d[:, b0 + off:b0 + off + fl, :])
                et = expp.tile([P, EG, CP], BF16, tag="exp")
                nc.scalar.activation(
                    et[:, 0:fl, :], pt[:, sl, :],
                    mybir.ActivationFunctionType.Exp,
                )
                c0 = b0 + off
                if fl <= 6:
                    # tiny tail fills: one flat reduce beats a sem-chained
                    # tree
                    nc.vector.tensor_reduce(
                        sume_t[:, c0:c0 + fl], et[:, 0:fl, :],
                        axis=mybir.AxisListType.X, op=ADD,
                    )
                else:
                    s1 = scrp.tile([P, EG, 160], BF16, tag="s1")
                    nc.vector.tensor_tensor(
                        s1[:, 0:fl, :], et[:, 0:fl, 0:160],
                        et[:, 0:fl, 160:320], op=ADD)
                    s2 = scrp.tile([P, EG, 80], BF16, tag="s2")
                    nc.vector.tensor_tensor(
                        s2[:, 0:fl, :], s1[:, 0:fl, 0:80], s1[:, 0:fl, 80:160],
                        op=ADD)
                    s3 = scrp.tile([P, EG, 40], BF16, tag="s3")
                    nc.vector.tensor_tensor(
                        s3[:, 0:fl, :], s2[:, 0:fl, 0:40], s2[:, 0:fl, 40:80],
                        op=ADD)
                    s4 = scrp.tile([P, EG, 20], BF16, tag="s4")
                    nc.vector.tensor_tensor(
                        s4[:, 0:fl, :], s3[:, 0:fl, 0:20], s3[:, 0:fl, 20:40],
                        op=ADD)
                    nc.vector.tensor_reduce(
                        sume_t[:, c0:c0 + fl], s4[:, 0:fl, :],
                        axis=mybir.AxisListType.X, op=ADD,
                    )

                # DoubleRow fp8: one matmul contracts a PAIR of chunks
                # (256 px). Class counts are even, so pairs never straddle
                # classes.
                for j in range(off, off + fl, 2):
                    ch = b0 + j
                    if ch >= nreal:
                        continue
                    m = cls_of(ch)
                    sz = CLS_SZ[m]
                    nc.tensor.matmul(
                        spsum[m][0:sz, :],
                        oht[:, j:j + 2, 0:sz],
                        pt[:, j:j + 2, 0:C],
                        start=(ch == starts[m]),
                        stop=(ch >= ends[m] - 2),
                        perf_mode=mybir.MatmulPerfMode.DoubleRow,
                    )
                off += fl
                n_emitted += fl
                for m in range(3):
                    if not closed[m] and n_emitted >= ends[m]:
                        emit_gdot(m)
                        closed[m] = True

        for m in range(3):
            if not closed[m]:
                emit_gdot(m)

        lse_t = accp.tile([P, nchunk], F32, tag="lse")
        nc.scalar.activation(
            lse_t[:], sume_t[:], mybir.ActivationFunctionType.Ln
        )
        scrl = accp.tile([P, nchunk], F32, tag="scrl")
        nc.vector.tensor_tensor(scrl[:], lse_t[:], coef_t[:], op=MULT)
        nc.vector.tensor_reduce(
            out_t[:, 0:1], scrl[:], axis=mybir.AxisListType.X, op=ADD
        )
        nc.sync.dma_start(out_d, out_t[:])

    nc.compile()
    return nc


def host_inputs(pred, binned_color, knn_idx, knn_weights, weights):
    """Returns (in_maps, ncls, g, nchunk)."""
    import ml_dtypes

    f8 = ml_dtypes.float8_e4m3
    pred = np.asarray(pred, dtype=np.float32)
    binned = np.asarray(binned_color)
    knn_idx = np.asarray(knn_idx).astype(np.int64)
    knn_w = np.asarray(knn_weights, dtype=np.float32)
    wts = np.asarray(weights, dtype=np.float32)

    # A[t, c] = w[t] * sum_k knn_w[t,k] * [knn_idx[t,k] == c]
    a_full = np.zeros((C, C), dtype=np.float32)
    rows = np.repeat(np.arange(C), K)
    cols = knn_idx.reshape(-1)
    vals = (wts[:, None] * knn_w).reshape(-1)
    np.add.at(a_full, (rows, cols), vals)
    atab = np.zeros((P, 3, C), dtype=np.float32)
    for m in range(3):
        atab[0:CLS_SZ[m], m, :] = a_full[CLS_OFF[m]:CLS_OFF[m] + CLS_SZ[m], :]

    coef_full = wts * knn_w.sum(axis=1)          # (C,)

    t_all, cls_all, counts_all = [], [], []
    for core in range(NCORES):
        bs = slice(core * BPC, (core + 1) * BPC)
        t = binned[bs, 0].reshape(PIX).astype(np.int64)
        cl = t // P
        t_all.append(t)
        cls_all.append(cl)
        counts_all.append(np.bincount(cl, minlength=3))
    counts_all = np.stack(counts_all)            # (8, 3)
    # even chunk counts per class so DoubleRow pairs stay within one class
    ncls = tuple(
        (int(-(-counts_all[:, m].max() // P)) + 1) // 2 * 2 for m in range(3)
    )
    nchunk, blocks = _plan(sum(ncls))
    base = np.cumsum([0, ncls[0], ncls[1]])[:3] * P

    in_maps = []
    for core in range(NCORES):
        bs = slice(core * BPC, (core + 1) * BPC)
        t, cl, cnt = t_all[core], cls_all[core], counts_all[core]
        order = np.argsort(cl, kind="stable")
        slots = np.empty(PIX, dtype=np.int64)
        pos = 0
        for m in range(3):
            slots[order[pos:pos + cnt[m]]] = base[m] + np.arange(cnt[m])
            pos += cnt[m]

        pm = np.ascontiguousarray(
            pred[bs].transpose(0, 2, 3, 1)
        ).reshape(PIX, C)
        pred_slots = np.zeros((nchunk * P, CP), dtype=f8)
        pred_slots[:, C:CP] = f8(-240.0)
        pred_slots[slots, 0:C] = pm.astype(f8)
        pred_pm = np.ascontiguousarray(
            pred_slots.reshape(nchunk, P, CP).transpose(1, 0, 2)
        )

        oh_u8 = np.zeros((nchunk * P, P), dtype=np.uint8)
        oh_u8[slots, t - cl * P] = 0x38          # fp8e4 1.0
        oh = np.ascontiguousarray(
            oh_u8.reshape(nchunk, P, P).transpose(1, 0, 2)
        ).view(f8)

        coef_slots = np.zeros(nchunk * P, dtype=np.float32)
        coef_slots[slots] = coef_full[t]
        coef = np.ascontiguousarray(coef_slots.reshape(nchunk, P).T)

        in_maps.append(
            {"pred_pm": pred_pm, "oh": oh, "coef": coef, "atab": atab}
        )
    return in_maps, ncls, nchunk, blocks


def combine_outputs(core_outs):
    """core_outs: list of [128, 4] f32 arrays -> scalar loss."""
    total = 0.0
    for o in core_outs:
        o = o.astype(np.float64)
        total += o[:, 0].sum() - o[:, 1:4].sum()
    return np.array(total / NTOT, dtype=np.float32)


_NC_CACHE = {}


def kernel(pred, _color, binned_color, knn_idx, knn_weights, weights):
    in_maps, ncls, nchunk, blocks = host_inputs(
        pred, binned_color, knn_idx, knn_weights, weights
    )
    key = (ncls, nchunk)
    if key not in _NC_CACHE:
        _NC_CACHE[key] = build_program(ncls, nchunk, blocks)
    nc = _NC_CACHE[key]
    res = bass_utils.run_bass_kernel_spmd(nc, in_maps, core_ids=list(range(NCORES)))
    outs = [res.results[i]["out"] for i in range(NCORES)]
    return combine_outputs(outs)


if __name__ == "__main__":
    import jax
    import reference

    with jax.default_device(jax.devices("cpu")[0]):
        inputs = reference.setup_inputs()
        inputs = {k: np.asarray(jax.device_get(v)) for k, v in inputs.items()}
    got = kernel(**inputs)
    print("kernel loss:", got)
